# revision 7
# baseline (speedup 1.0000x reference)
"""FPQuantLinear (MXFP4 pseudo-quant linear) Trainium2 kernel.

y = einsum('bsk,nk->bsn', Q(x), Q(w)) + bias
where Q = per-32-group Hadamard rotation + MXFP4 (e2m1 + power-of-2 block
scale) quant-dequant with abs_max scaling and a global scale.

Strategy (8 NeuronCores, token-parallel):
  - tokens (B*S = 8192) sharded 1024/core; W replicated (V1) or shard-quantized.
  - Host marshals inputs: H = s*P with P = +-1 (exact in fp16); x and w are
    pre-scaled by (global_scale * s), split into two fp16 halves (hi/lo) so the
    PE computes the fp32-accurate rotated-scaled tensor v = rot * gs via 2
    fp16 matmuls vs the exact +-1 block-diagonal pattern. Host also
    pre-transposes to K-major so no on-device transposes are needed.
  - Per 32-group scale = 2^ceil(log2(absmax/6 + 1e-30)) computed with integer
    exponent tricks on DVE; absmax over the 32 partitions of each k-group via
    gpsimd.partition_all_reduce (or DVE transpose32 fallback).
  - e2m1 round-to-nearest: r = (v + cs) - cs with cs = sign(v)|C and
    C = 1.5 * 2^22 * max(expfloat(v), scale)  -> r = dq * gs exactly, which is
    exactly representable in fp8e4m3 for this data distribution.
  - Main matmul in fp8: yT[n, t] accumulated over k in PSUM (fp32), then
    out = yT * (1/(gs_x*gs_w)) + bias via ACT, DMA to DRAM; host transposes
    shards back and reassembles [4, 2048, 4096] fp32.
"""
import os

import numpy as np

GROUP = 32
B, S, K, N = 4, 2048, 4096, 4096
TOK = B * S
NCORES = 8
TPC = TOK // NCORES  # tokens per core

_prog_cache = {}
LAST_EXEC_NS = None
LAST_RESULTS = None


def _build_program(n_splits, use_gpsimd_absmax, w_shard):
    import concourse.bass as bass
    import concourse.mybir as mybir
    import concourse.tile as tile
    from concourse import bacc, bass_isa

    F32 = mybir.dt.float32
    F16 = mybir.dt.float16
    FP8 = mybir.dt.float8e4
    I32 = mybir.dt.int32
    Alu = mybir.AluOpType
    Act = mybir.ActivationFunctionType

    KB = K // 128          # 32 k-blocks
    NT = N // 128          # 32 n-tiles
    FDC = 1024             # quant chunk free-dim
    WCH = N // FDC         # 4 n-chunks per k-block in W quant
    assert not w_shard, "collective path not implemented yet"

    nc = bacc.Bacc(None, target_bir_lowering=False)

    xh_d = nc.dram_tensor("xhT", [K, TPC], F16, kind="ExternalInput")
    xl_d = nc.dram_tensor("xlT", [K, TPC], F16, kind="ExternalInput")
    wh_d = nc.dram_tensor("whT", [K, N], F16, kind="ExternalInput")
    wl_d = nc.dram_tensor("wlT", [K, N], F16, kind="ExternalInput")
    bdp_d = nc.dram_tensor("bdp", [n_splits, 128, 128], F16, kind="ExternalInput")
    bias_d = nc.dram_tensor("bias", [N], F32, kind="ExternalInput")
    scl_d = nc.dram_tensor("scl", [128, 1], F32, kind="ExternalInput")  # 1/(gsx*gsw)
    y_d = nc.dram_tensor("yT", [N, TPC], F32, kind="ExternalOutput")

    dbg = os.environ.get("KQ_DEBUG", "0") == "1"
    if dbg:
        wdq_d = nc.dram_tensor("wdq_i", [KB, 128, N], FP8, kind="ExternalOutput")
    else:
        wdq_d = nc.dram_tensor("wdq_i", [KB, 128, N], FP8)
    xdq_d = nc.dram_tensor("xdq_i", [128, KB, TPC], FP8,
                           kind="ExternalOutput") if dbg else None

    with tile.TileContext(nc) as tc:
        with (
            tc.tile_pool(name="singles", bufs=1) as singles,
            tc.tile_pool(name="stage", bufs=3) as stage,
            tc.tile_pool(name="qwork", bufs=2) as qwork,
            tc.tile_pool(name="dqout", bufs=3) as dqout,
            tc.tile_pool(name="wmain", bufs=3) as wmain,
            tc.tile_pool(name="ymain", bufs=3) as ymain,
            tc.tile_pool(name="vps", bufs=2, space="PSUM") as vps,
            tc.tile_pool(name="yps", bufs=2, space="PSUM") as yps,
        ):
            # ---------- constants ----------
            bdp_t = singles.tile([128, n_splits, 128], F16)
            for sp in range(n_splits):
                nc.sync.dma_start(bdp_t[:, sp, :], bdp_d[sp])
            bias_t = singles.tile([128, NT], F32)
            nc.sync.dma_start(bias_t[:], bias_d[:].rearrange("(a b) -> b a", b=128))
            scl_bc = singles.tile([128, 1], F32)
            nc.sync.dma_start(scl_bc[:], scl_d[:])
            sgnmask = singles.tile([128, 1], I32)
            nc.vector.memset(sgnmask[:], -0x80000000)
            xdq_t = singles.tile([128, KB, TPC], FP8)

            inv6 = float(np.float32(1.0) / np.float32(6.0))

            # ---------- quantization pipeline for one [128, fd] chunk ----------
            def quant_chunk(h_t, l_t, fd, dq_out_ap):
                vp = vps.tile([128, FDC], F32, tag="vp")
                for j in range(fd // 512):
                    sl = slice(j * 512, (j + 1) * 512)
                    for sp in range(n_splits):
                        nc.tensor.matmul(
                            vp[:, sl], bdp_t[:, sp, :], (h_t, l_t)[sp % 2][:, sl],
                            start=(sp == 0), stop=(sp == n_splits - 1),
                        )
                vc = qwork.tile([128, FDC], F32, tag="vc")
                nc.scalar.activation(vc[:, :fd], vp[:, :fd], Act.Copy)
                ab = qwork.tile([128, FDC], F32, tag="ab")
                if use_gpsimd_absmax:
                    for g in range(4):
                        sl = slice(g * 32, (g + 1) * 32)
                        nc.gpsimd.partition_all_reduce(
                            ab[sl, :fd], vc[sl, :fd], channels=32,
                            reduce_op=bass_isa.ReduceOp.absmax,
                        )
                else:
                    t32 = qwork.tile([128, FDC], F32, tag="t32")
                    nc.vector.transpose(t32[:, :fd], vc[:, :fd])
                    red = qwork.tile([128, FDC // 32], F32, tag="red")
                    nc.vector.tensor_reduce(
                        red[:, : fd // 32],
                        t32[:, :fd].rearrange("p (j b) -> p j b", b=32),
                        mybir.AxisListType.X, Alu.max,
                        apply_absolute_value=True,
                    )
                    exp32 = qwork.tile([128, FDC], F32, tag="exp32")
                    nc.vector.tensor_copy(
                        exp32[:, :fd].rearrange("p (j b) -> p j b", b=32),
                        red[:, : fd // 32].unsqueeze(2).broadcast_to(
                            [128, fd // 32, 32]
                        ),
                    )
                    nc.vector.transpose(ab[:, :fd], exp32[:, :fd])
                # t2 = ab*inv6 + 1e-30 ; sc = exp2(ceil(log2(t2))) via int trick
                t2 = qwork.tile([128, FDC], F32, tag="t2")
                nc.vector.tensor_scalar(
                    t2[:, :fd], ab[:, :fd], inv6, 1e-30, Alu.mult, Alu.add
                )
                u1 = qwork.tile([128, FDC], F32, tag="u1")
                nc.vector.tensor_scalar(
                    u1[:, :fd].bitcast(I32), t2[:, :fd].bitcast(I32),
                    0x7FFFFF, None, Alu.add,
                )
                sc = qwork.tile([128, FDC], F32, tag="sc")
                nc.vector.tensor_scalar(
                    sc[:, :fd].bitcast(I32), u1[:, :fd].bitcast(I32),
                    0x7F800000, None, Alu.bitwise_and,
                )
                # e = max(expbits(v), sc_bits) ; C = e + 0x0B400000
                ea = qwork.tile([128, FDC], F32, tag="ea")
                nc.vector.tensor_scalar(
                    ea[:, :fd].bitcast(I32), vc[:, :fd].bitcast(I32),
                    0x7F800000, None, Alu.bitwise_and,
                )
                e = qwork.tile([128, FDC], F32, tag="e")
                nc.vector.tensor_tensor(
                    e[:, :fd].bitcast(I32), ea[:, :fd].bitcast(I32),
                    sc[:, :fd].bitcast(I32), Alu.max,
                )
                C = qwork.tile([128, FDC], F32, tag="C")
                nc.vector.tensor_scalar(
                    C[:, :fd].bitcast(I32), e[:, :fd].bitcast(I32),
                    0x0B400000, None, Alu.add,
                )
                cs = qwork.tile([128, FDC], F32, tag="cs")
                nc.vector.scalar_tensor_tensor(
                    cs[:, :fd].bitcast(I32), vc[:, :fd].bitcast(I32),
                    sgnmask[:, 0:1], C[:, :fd].bitcast(I32),
                    Alu.bitwise_and, Alu.bitwise_or,
                )
                t = qwork.tile([128, FDC], F32, tag="t")
                nc.vector.tensor_tensor(t[:, :fd], vc[:, :fd], cs[:, :fd], Alu.add)
                nc.vector.tensor_tensor(dq_out_ap, t[:, :fd], cs[:, :fd], Alu.subtract)

            # ---------- phase 1: quantize W ----------
            for kb in range(KB):
                for ch in range(WCH):
                    nsl = slice(ch * FDC, (ch + 1) * FDC)
                    h_t = stage.tile([128, FDC], F16, tag="wh")
                    l_t = stage.tile([128, FDC], F16, tag="wl")
                    nc.sync.dma_start(h_t[:], wh_d[kb * 128:(kb + 1) * 128, nsl])
                    nc.sync.dma_start(l_t[:], wl_d[kb * 128:(kb + 1) * 128, nsl])
                    dq_t = dqout.tile([128, FDC], FP8, tag="wdq")
                    quant_chunk(h_t, l_t, FDC, dq_t[:])
                    nc.sync.dma_start(wdq_d[kb, :, nsl], dq_t[:])

            # ---------- phase 2: quantize x shard ----------
            for kb in range(KB):
                h_t = stage.tile([128, FDC], F16, tag="xh")
                l_t = stage.tile([128, FDC], F16, tag="xl")
                nc.sync.dma_start(h_t[:, :TPC], xh_d[kb * 128:(kb + 1) * 128, :])
                nc.sync.dma_start(l_t[:, :TPC], xl_d[kb * 128:(kb + 1) * 128, :])
                quant_chunk(h_t, l_t, TPC, xdq_t[:, kb, :])

            if dbg:
                nc.sync.dma_start(xdq_d[:], xdq_t[:])

            # ---------- phase 3: main matmul yT = wdq^T @ xdq ----------
            for nt in range(NT):
                wnt = wmain.tile([128, KB, 128], FP8, tag="wnt")
                nc.sync.dma_start(
                    wnt[:],
                    wdq_d[:, :, nt * 128:(nt + 1) * 128].rearrange(
                        "kb p n -> p kb n"
                    ),
                )
                for tch in range(TPC // 512):
                    tsl = slice(tch * 512, (tch + 1) * 512)
                    yp = yps.tile([128, 512], F32, tag="yp")
                    for kb in range(KB):
                        nc.tensor.matmul(
                            yp[:], wnt[:, kb, :], xdq_t[:, kb, tsl],
                            start=(kb == 0), stop=(kb == KB - 1),
                        )
                    ysb = ymain.tile([128, 512], F32, tag="ysb")
                    nc.scalar.activation(
                        ysb[:], yp[:], Act.Identity,
                        bias=bias_t[:, nt:nt + 1], scale=scl_bc[:, 0:1],
                    )
                    nc.sync.dma_start(y_d[nt * 128:(nt + 1) * 128, tsl], ysb[:])

    nc.compile()
    return nc


def _get_program(n_splits):
    use_gpsimd = os.environ.get("KQ_GPSIMD", "1") == "1"
    key = (n_splits, use_gpsimd, os.environ.get("KQ_DEBUG", "0"))
    if key not in _prog_cache:
        _prog_cache[key] = _build_program(n_splits, use_gpsimd, False)
    return _prog_cache[key]


def _prepare(x, weight, bias, hadamard_matrix, weight_global_scale, act_global_scale):
    x = np.asarray(x, dtype=np.float32)
    weight = np.asarray(weight, dtype=np.float32)
    bias = np.asarray(bias, dtype=np.float32)
    H = np.asarray(hadamard_matrix, dtype=np.float32)
    gsw = np.float32(np.asarray(weight_global_scale).reshape(-1)[0])
    gsx = np.float32(np.asarray(act_global_scale).reshape(-1)[0])

    s = np.float32(np.abs(H).max())
    Pm = (H / s).astype(np.float32)
    Ph = Pm.astype(np.float16)
    Pl = (Pm - Ph.astype(np.float32)).astype(np.float16)
    generic = bool(np.any(Pl != 0))
    n_splits = 4 if generic else 2

    eye4 = np.eye(4, dtype=np.float32)
    if generic:
        # (xh+xl) @ (Ph+Pl): splits (xh@Ph, xl@Ph, xh@Pl, xl@Pl)
        bdp = np.stack([
            np.kron(eye4, Ph.astype(np.float32)).astype(np.float16),
            np.kron(eye4, Ph.astype(np.float32)).astype(np.float16),
            np.kron(eye4, Pl.astype(np.float32)).astype(np.float16),
            np.kron(eye4, Pl.astype(np.float32)).astype(np.float16),
        ])
    else:
        bdp = np.stack([np.kron(eye4, Ph.astype(np.float32)).astype(np.float16)] * 2)

    cx = np.float32(gsx * s)
    cw = np.float32(gsw * s)
    inv_gs = np.full((128, 1), np.float32(1.0) / np.float32(gsx * gsw), dtype=np.float32)

    xs = (x.reshape(TOK, K) * cx).astype(np.float32)
    xh = xs.astype(np.float16)
    xl = (xs - xh.astype(np.float32)).astype(np.float16)

    ws = (weight * cw).astype(np.float32)
    wh = ws.astype(np.float16)
    wl = (ws - wh.astype(np.float32)).astype(np.float16)
    whT = np.ascontiguousarray(wh.T)
    wlT = np.ascontiguousarray(wl.T)

    nc = _get_program(n_splits)

    in_maps = []
    for c in range(NCORES):
        tsl = slice(c * TPC, (c + 1) * TPC)
        in_maps.append({
            "xhT": np.ascontiguousarray(xh[tsl].T),
            "xlT": np.ascontiguousarray(xl[tsl].T),
            "whT": whT,
            "wlT": wlT,
            "bdp": bdp,
            "bias": bias,
            "scl": inv_gs,
        })

    return nc, in_maps


def _assemble(results):
    y = np.empty((TOK, N), dtype=np.float32)
    for c in range(NCORES):
        y[c * TPC:(c + 1) * TPC] = results[c]["yT"].T
    return y.reshape(B, S, N)


def kernel(x, weight, bias, hadamard_matrix, weight_global_scale, act_global_scale):
    from concourse.bass_utils import run_bass_kernel_spmd

    nc, in_maps = _prepare(x, weight, bias, hadamard_matrix,
                           weight_global_scale, act_global_scale)
    res = run_bass_kernel_spmd(nc, in_maps, list(range(NCORES)))
    global LAST_RESULTS
    LAST_RESULTS = res.results
    return _assemble(res.results)
